# revision 27
# baseline (speedup 1.0000x reference)
"""Multi-head attention (B=2, S=2048, D=1024, H=16) on 8 NeuronCores.

Sharding: core = b*4 + g  (b = batch 0..1, g = head-group 0..3, 4 heads each).
Per core, everything is computed in transposed layouts so no on-device
transposes are needed:
  qT,kT = W.T @ x.T   (via matmul lhsT=W, rhs=xT)            [256, 2048]
  v     = x @ Wv      (normal layout, + ones column)          [2048, 4, 65]
  ST    = kT.T @ qT   (scores, transposed: [sk, sq] tiles)
  P^T   = exp(ST + tri-mask)      (elementwise, no transpose needed)
  outT  = [v|1].T @ P^T           (ones column -> softmax sums for free)
  ctxT  = outT[0:64] * (1/outT[64])   (broadcast via PE ones-matmul)
  yT    = Wo.T-slice @ ctxT + bo  (partial over head group)   [1024, 2048]
Host sums the 4 partial yT per batch and transposes.

Matmuls run as float32r (fp32 bits, TF32-like PE fast path, ~1.5e-4 rel err).
Causal masking: only sk-tiles t <= 4c+3 are computed; the diagonal tile's
boundary strip gets an additive lower-triangle mask on DVE, fully-masked
column prefixes are memset to 0 on GpSimd and skipped by the exp.
"""

import numpy as np

import concourse.mybir as mybir
import concourse.tile as tile
from concourse import bacc
from concourse.bass_utils import run_bass_kernel_spmd

B, S, D, H = 2, 2048, 1024, 16
DK = 64
NCORES = 8
HPC = 4                      # heads per core
DH = HPC * DK                # 256 per-core head dims
CH = 512                     # sq chunk size
NCH = S // CH                # 4 chunks
NST = S // 128               # 16 s-tiles
NKD = D // 128               # 8 contraction tiles over D
F32 = mybir.dt.float32
F32R = mybir.dt.float32r
MASK_NEG = -100.0


def build_program(has_bias: bool):
    nc = bacc.Bacc("TRN2", target_bir_lowering=False, debug=False)

    xT = nc.dram_tensor("xT", [D, S], F32R, kind="ExternalInput")
    wq = nc.dram_tensor("wq", [D, DH], F32R, kind="ExternalInput")
    wk = nc.dram_tensor("wk", [D, DH], F32R, kind="ExternalInput")
    wv = nc.dram_tensor("wv", [D, DH], F32R, kind="ExternalInput")
    wo = nc.dram_tensor("wo", [DH, D], F32R, kind="ExternalInput")
    bq = nc.dram_tensor("bq", [DH], F32R, kind="ExternalInput")
    bk = nc.dram_tensor("bk", [DH], F32R, kind="ExternalInput")
    bv = nc.dram_tensor("bv", [DH], F32R, kind="ExternalInput")
    bo = nc.dram_tensor("bo", [D], F32, kind="ExternalInput")
    trimask = nc.dram_tensor("trimask", [128, 128], F32, kind="ExternalInput")
    ones_row = nc.dram_tensor("ones_row", [1, CH], F32R, kind="ExternalInput")
    ones4 = nc.dram_tensor("ones4", [128, HPC, 1], F32R, kind="ExternalInput")
    outT = nc.dram_tensor("outT", [D, S], F32, kind="ExternalOutput")

    Exp = mybir.ActivationFunctionType.Exp
    Copy = mybir.ActivationFunctionType.Copy

    with tile.TileContext(nc) as tc:
        with (
            tc.tile_pool(name="persist", bufs=1) as pp,
            tc.tile_pool(name="work", bufs=6) as wp,
            tc.tile_pool(name="pt_pool", bufs=6) as ptp,
            tc.tile_pool(name="small", bufs=3) as sp,
            tc.tile_pool(name="xsl", bufs=2) as xp,
            tc.tile_pool(name="psum", bufs=2, space="PSUM") as ps,
        ):
            # ---- SBUF tensors needed by phase A chunk 0, DMA'd first ----
            wq_sb = pp.tile([128, NKD, DH], F32R, tag="wq")
            wk_sb = pp.tile([128, NKD, DH], F32R, tag="wk")
            wv_sb = pp.tile([128, NKD, DH], F32R, tag="wv")
            wq_ap = wq.ap().rearrange("(t p) n -> p t n", p=128)
            wk_ap = wk.ap().rearrange("(t p) n -> p t n", p=128)
            wv_ap = wv.ap().rearrange("(t p) n -> p t n", p=128)

            qt_sb = [pp.tile([128, S], F32R, tag=f"qt{m}", name=f"qt{m}") for m in range(2)]
            kt_sb = [pp.tile([128, S], F32R, tag=f"kt{m}", name=f"kt{m}") for m in range(2)]
            ctx_sb = [pp.tile([128, S], F32R, tag=f"ctx{m}", name=f"ctx{m}") for m in range(2)]
            vext = [pp.tile([128, HPC, DK + 1], F32R, tag=f"vext{i}", name=f"vext{i}") for i in range(NST)]

            ones_sb = pp.tile([1, CH], F32R, tag="ones")
            if has_bias:
                bq_sb = pp.tile([1, DH], F32R, tag="bq")
                bk_sb = pp.tile([1, DH], F32R, tag="bk")
                bv_sb = pp.tile([1, DH], F32R, tag="bv")

            def emit_late_dmas():
                """Inputs not needed until later; emitted after chunk 0."""
                nc.gpsimd.dma_start(out=mask_sb[:], in_=trimask.ap())
                nc.gpsimd.dma_start(out=wo_sb[:], in_=wo.ap().rearrange("(t p) n -> p t n", p=128))
                nc.gpsimd.dma_start(out=bo_sb[:], in_=bo.ap().rearrange("(t p) -> p t", p=128))
                nc.gpsimd.dma_start(out=ones_sb[:], in_=ones_row.ap())
                if has_bias:
                    nc.gpsimd.dma_start(out=bq_sb[:], in_=bq.ap()[None, :])
                    nc.gpsimd.dma_start(out=bk_sb[:], in_=bk.ap()[None, :])
                    nc.gpsimd.dma_start(out=bv_sb[:], in_=bv.ap()[None, :])
                for i in range(NST):
                    nc.gpsimd.dma_start(out=vext[i][:, :, DK : DK + 1], in_=ones4.ap())

            mask_sb = pp.tile([128, 128], F32, tag="mask")
            wo_sb = pp.tile([128, 2, D], F32R, tag="wo")
            bo_sb = pp.tile([128, NKD], F32, tag="bo")

            # ====== Phases A (projections), B (attention), C (out proj) ======
            # emitted interleaved (A0 A1 B0 C0 A2 B1 C1 A3 B2 C2 B3 C3) so the
            # exp-heavy (ACT-bound) B sections overlap the PE-bound A sections.
            # One unified PSUM pool: A's q/k accumulators share the "st" tag,
            # A's v accumulators share the "fo" tag with C's output tiles.


            def phase_a(c):
                xs = []
                for k in range(NKD):
                    t = xp.tile([128, CH], F32R, tag=f"xs{k}", name=f"xs{k}_{c}")
                    eng = nc.sync if k % 2 == 0 else nc.scalar
                    eng.dma_start(
                        out=t[:], in_=xT.ap()[k * 128 : (k + 1) * 128, c * CH : (c + 1) * CH]
                    )
                    xs.append(t)
                    if c == 0:
                        nc.sync.dma_start(out=wq_sb[:, k, :], in_=wq_ap[:, k, :])
                        nc.gpsimd.dma_start(out=wk_sb[:, k, :], in_=wk_ap[:, k, :])
                if c == 0:
                    for k in range(NKD):
                        nc.gpsimd.dma_start(out=wv_sb[:, k, :], in_=wv_ap[:, k, :])
                for name, w_sb, dst, scale in (
                    ("q", wq_sb, qt_sb, None),
                    ("k", wk_sb, kt_sb, 0.125),
                ):
                    for m in range(2):
                        ps_t = ps.tile([128, CH], F32, tag="st", bufs=4, name=f"{name}ps{m}_{c}")
                        for k in range(NKD):
                            nc.tensor.matmul(
                                ps_t[:],
                                w_sb[:, k, m * 128 : (m + 1) * 128],
                                xs[k][:],
                                start=(k == 0),
                                stop=(not has_bias and k == NKD - 1),
                            )
                        if has_bias:
                            b_sb = bq_sb if name == "q" else bk_sb
                            nc.tensor.matmul(
                                ps_t[:],
                                b_sb[:, m * 128 : (m + 1) * 128],
                                ones_sb[:],
                                start=False,
                                stop=True,
                            )
                        dslice = dst[m][:, c * CH : (c + 1) * CH]
                        if scale is None:
                            nc.vector.tensor_copy(dslice, ps_t[:])
                        else:
                            nc.vector.tensor_scalar_mul(dslice, ps_t[:], scale)
                for i in range(4):
                    si = 4 * c + i
                    ps_t = ps.tile([128, DH], F32, tag="fo", bufs=2, name=f"vps_{si}")
                    for k in range(NKD):
                        nc.tensor.matmul(
                            ps_t[:],
                            xs[k][:, i * 128 : (i + 1) * 128],
                            wv_sb[:, k, :],
                            start=(k == 0),
                            stop=(not has_bias and k == NKD - 1),
                        )
                    if has_bias:
                        nc.tensor.matmul(
                            ps_t[:], ones_sb[:, 0:128], bv_sb[:], start=False, stop=True
                        )
                    nc.vector.tensor_copy(
                        vext[si][:, :, 0:DK],
                        ps_t[:].rearrange("p (h d) -> p h d", h=HPC),
                    )
                if c == 0:
                    emit_late_dmas()

            def phase_b(c):
                nk = 4 * c + 4  # causal: sk-tiles 0 .. 4c+3
                for hp in (0, 2):  # interleave two heads per stream
                    pair = (hp, hp + 1)
                    avs = {}
                    for h in pair:
                        avs[h] = ps.tile(
                            [DK + 1, CH], F32, tag=f"av{h % 2}", bufs=1, name=f"av{h}_{c}"
                        )
                    for t in range(nk):
                        o = t - 4 * c
                        pts = {}
                        for h in pair:
                            m, r0 = h // 2, (h % 2) * DK
                            st = ps.tile(
                                [128, CH], F32, tag="st", bufs=4, name=f"st{h}_{c}_{t}"
                            )
                            nc.tensor.matmul(
                                st[:],
                                kt_sb[m][r0 : r0 + DK, t * 128 : (t + 1) * 128],
                                qt_sb[m][r0 : r0 + DK, c * CH : (c + 1) * CH],
                                start=True,
                                stop=True,
                            )
                            pt = ptp.tile([128, CH], F32R, tag="pt", name=f"pt{h}_{c}_{t}")
                            if o >= 0:
                                nc.vector.tensor_add(
                                    st[:, 128 * o : 128 * (o + 1)],
                                    st[:, 128 * o : 128 * (o + 1)],
                                    mask_sb[:],
                                )
                            z = 128 * max(o, 0)
                            nc.scalar.activation(
                                out=pt[:, z:CH], in_=st[:, z:CH], func=Exp
                            )
                            pts[h] = (pt, z)
                        for h in pair:
                            pt, z = pts[h]
                            nc.tensor.matmul(
                                avs[h][:, z:CH],
                                vext[t][:, h, :],
                                pt[:, z:CH],
                                start=(t == 0),
                                stop=(t == nk - 1),
                            )
                    for h in pair:
                        m, r0 = h // 2, (h % 2) * DK
                        av = avs[h]
                        recip = sp.tile([1, CH], F32, tag="recip", name=f"recip{h}_{c}")
                        nc.vector.reciprocal(recip[:], av[DK : DK + 1, :])
                        bc_sb = sp.tile([DK, CH], F32, tag="bcsb", name=f"bcsb{h}_{c}")
                        nc.gpsimd.partition_broadcast(bc_sb[:], recip[:])
                        nc.vector.tensor_mul(
                            ctx_sb[m][r0 : r0 + DK, c * CH : (c + 1) * CH],
                            av[0:DK, :],
                            bc_sb[:],
                        )

            def phase_c(c):
                for dt in range(NKD):
                    fo = ps.tile([128, CH], F32, tag="fo", bufs=2, name=f"fo{dt}_{c}")
                    for kk in range(2):
                        nc.tensor.matmul(
                            fo[:],
                            wo_sb[:, kk, dt * 128 : (dt + 1) * 128],
                            ctx_sb[kk][:, c * CH : (c + 1) * CH],
                            start=(kk == 0),
                            stop=(kk == 1),
                        )
                    ob = wp.tile([128, CH], F32, tag="ob", name=f"ob{dt}_{c}")
                    nc.vector.tensor_scalar_add(ob[:], fo[:], bo_sb[:, dt : dt + 1])
                    nc.scalar.dma_start(
                        out=outT.ap()[dt * 128 : (dt + 1) * 128, c * CH : (c + 1) * CH],
                        in_=ob[:],
                    )

            phase_a(0)
            phase_a(1)
            for c in range(NCH):
                if c + 2 < NCH:
                    phase_a(c + 2)
                phase_b(c)
                phase_c(c)


    nc.finalize()
    return nc


_NC_CACHE = {}


def get_nc(has_bias: bool = True):
    key = ("nc", has_bias)
    if key not in _NC_CACHE:
        _NC_CACHE[key] = build_program(has_bias)
    return _NC_CACHE[key]


def make_in_maps(x, mask, Wq, bq, Wk, bk, Wv, bv, Wo, bo):
    x = np.asarray(x, dtype=np.float32)
    mask = np.asarray(mask)
    Wq, Wk, Wv, Wo = (np.asarray(w, dtype=np.float32) for w in (Wq, Wk, Wv, Wo))
    bq, bk, bv, bo = (np.asarray(v, dtype=np.float32) for v in (bq, bk, bv, bo))

    # lower-triangle-complement boundary strip: masked iff key j > query i
    # within a diagonal 128x128 block -> additive MASK_NEG where mask==1
    trimask = np.where(mask[0:128, 0:128].T == 1, MASK_NEG, 0.0).astype(np.float32)
    ones_row = np.ones((1, CH), dtype=np.float32)
    ones4 = np.ones((128, HPC, 1), dtype=np.float32)
    zeros_bo = np.zeros_like(bo)

    in_maps = []
    for core in range(NCORES):
        b, g = core // 4, core % 4
        sl = slice(g * DH, (g + 1) * DH)
        in_maps.append(
            {
                "xT": np.ascontiguousarray(x[b].T),
                "wq": np.ascontiguousarray(Wq[:, sl]),
                "wk": np.ascontiguousarray(Wk[:, sl]),
                "wv": np.ascontiguousarray(Wv[:, sl]),
                "wo": np.ascontiguousarray(Wo[sl, :]),
                "bq": np.ascontiguousarray(bq[sl]),
                "bk": np.ascontiguousarray(bk[sl]),
                "bv": np.ascontiguousarray(bv[sl]),
                "bo": bo if g == 0 else zeros_bo,
                "trimask": trimask,
                "ones_row": ones_row,
                "ones4": ones4,
            }
        )
    return in_maps


def gather_output(results):
    out = np.empty((B, S, D), dtype=np.float32)
    for b in range(B):
        acc = results[b * 4]["outT"].copy()
        for g in range(1, 4):
            acc += results[b * 4 + g]["outT"]
        out[b] = acc.T
    return out


def kernel(x, mask, Wq, bq, Wk, bk, Wv, bv, Wo, bo):
    has_bias = any(
        np.asarray(v, dtype=np.float32).any() for v in (bq, bk, bv)
    )
    nc = get_nc(has_bias)
    in_maps = make_in_maps(x, mask, Wq, bq, Wk, bk, Wv, bv, Wo, bo)
    res = run_bass_kernel_spmd(nc, in_maps, list(range(NCORES)))
    return gather_output(res.results)


# revision 28
# speedup vs baseline: 1.0396x; 1.0396x over previous
"""Multi-head attention (B=2, S=2048, D=1024, H=16) on 8 NeuronCores.

Sharding: core = b*4 + g  (b = batch 0..1, g = head-group 0..3, 4 heads each).
Per core, everything is computed in transposed layouts so no on-device
transposes are needed:
  qT,kT = W.T @ x.T   (via matmul lhsT=W, rhs=xT)            [256, 2048]
  v     = x @ Wv      (normal layout, + ones column)          [2048, 4, 65]
  ST    = kT.T @ qT   (scores, transposed: [sk, sq] tiles)
  P^T   = exp(ST + tri-mask)      (elementwise, no transpose needed)
  outT  = [v|1].T @ P^T           (ones column -> softmax sums for free)
  ctxT  = outT[0:64] * (1/outT[64])   (broadcast via PE ones-matmul)
  yT    = Wo.T-slice @ ctxT + bo  (partial over head group)   [1024, 2048]
Host sums the 4 partial yT per batch and transposes.

Matmuls run as float32r (fp32 bits, TF32-like PE fast path, ~1.5e-4 rel err).
Causal masking: only sk-tiles t <= 4c+3 are computed; the diagonal tile's
boundary strip gets an additive lower-triangle mask on DVE, fully-masked
column prefixes are memset to 0 on GpSimd and skipped by the exp.
"""

import numpy as np

import concourse.mybir as mybir
import concourse.tile as tile
from concourse import bacc

B, S, D, H = 2, 2048, 1024, 16
DK = 64
NCORES = 8
HPC = 4                      # heads per core
DH = HPC * DK                # 256 per-core head dims
CH = 512                     # sq chunk size
NCH = S // CH                # 4 chunks
NST = S // 128               # 16 s-tiles
NKD = D // 128               # 8 contraction tiles over D
F32 = mybir.dt.float32
F32R = mybir.dt.float32r
MASK_NEG = -100.0


def build_program(has_bias: bool):
    nc = bacc.Bacc("TRN2", target_bir_lowering=False, debug=False)

    xT = nc.dram_tensor("xT", [D, S], F32R, kind="ExternalInput")
    wq = nc.dram_tensor("wq", [D, DH], F32R, kind="ExternalInput")
    wk = nc.dram_tensor("wk", [D, DH], F32R, kind="ExternalInput")
    wv = nc.dram_tensor("wv", [D, DH], F32R, kind="ExternalInput")
    wo = nc.dram_tensor("wo", [DH, D], F32R, kind="ExternalInput")
    bq = nc.dram_tensor("bq", [DH], F32R, kind="ExternalInput")
    bk = nc.dram_tensor("bk", [DH], F32R, kind="ExternalInput")
    bv = nc.dram_tensor("bv", [DH], F32R, kind="ExternalInput")
    bo = nc.dram_tensor("bo", [D], F32, kind="ExternalInput")
    trimask = nc.dram_tensor("trimask", [128, 128], F32, kind="ExternalInput")
    ones_row = nc.dram_tensor("ones_row", [1, CH], F32R, kind="ExternalInput")
    ones4 = nc.dram_tensor("ones4", [128, HPC, 1], F32R, kind="ExternalInput")
    outT = nc.dram_tensor("outT", [D, S], F32, kind="ExternalOutput")

    Exp = mybir.ActivationFunctionType.Exp
    Copy = mybir.ActivationFunctionType.Copy

    with tile.TileContext(nc) as tc:
        with (
            tc.tile_pool(name="persist", bufs=1) as pp,
            tc.tile_pool(name="work", bufs=6) as wp,
            tc.tile_pool(name="pt_pool", bufs=6) as ptp,
            tc.tile_pool(name="small", bufs=3) as sp,
            tc.tile_pool(name="xsl", bufs=2) as xp,
            tc.tile_pool(name="psum", bufs=2, space="PSUM") as ps,
        ):
            # ---- SBUF tensors needed by phase A chunk 0, DMA'd first ----
            wq_sb = pp.tile([128, NKD, DH], F32R, tag="wq")
            wk_sb = pp.tile([128, NKD, DH], F32R, tag="wk")
            wv_sb = pp.tile([128, NKD, DH], F32R, tag="wv")
            wq_ap = wq.ap().rearrange("(t p) n -> p t n", p=128)
            wk_ap = wk.ap().rearrange("(t p) n -> p t n", p=128)
            wv_ap = wv.ap().rearrange("(t p) n -> p t n", p=128)

            qt_sb = [pp.tile([128, S], F32R, tag=f"qt{m}", name=f"qt{m}") for m in range(2)]
            kt_sb = [pp.tile([128, S], F32R, tag=f"kt{m}", name=f"kt{m}") for m in range(2)]
            ctx_sb = [pp.tile([128, S], F32R, tag=f"ctx{m}", name=f"ctx{m}") for m in range(2)]
            vext = [pp.tile([128, HPC, DK + 1], F32R, tag=f"vext{i}", name=f"vext{i}") for i in range(NST)]

            ones_sb = pp.tile([1, CH], F32R, tag="ones")
            if has_bias:
                bq_sb = pp.tile([1, DH], F32R, tag="bq")
                bk_sb = pp.tile([1, DH], F32R, tag="bk")
                bv_sb = pp.tile([1, DH], F32R, tag="bv")

            def emit_late_dmas():
                """Inputs not needed until later; emitted after chunk 0."""
                nc.gpsimd.dma_start(out=mask_sb[:], in_=trimask.ap())
                nc.gpsimd.dma_start(out=wo_sb[:], in_=wo.ap().rearrange("(t p) n -> p t n", p=128))
                nc.gpsimd.dma_start(out=bo_sb[:], in_=bo.ap().rearrange("(t p) -> p t", p=128))
                nc.gpsimd.dma_start(out=ones_sb[:], in_=ones_row.ap())
                if has_bias:
                    nc.gpsimd.dma_start(out=bq_sb[:], in_=bq.ap()[None, :])
                    nc.gpsimd.dma_start(out=bk_sb[:], in_=bk.ap()[None, :])
                    nc.gpsimd.dma_start(out=bv_sb[:], in_=bv.ap()[None, :])
                for i in range(NST):
                    nc.gpsimd.dma_start(out=vext[i][:, :, DK : DK + 1], in_=ones4.ap())

            mask_sb = pp.tile([128, 128], F32, tag="mask")
            wo_sb = pp.tile([128, 2, D], F32R, tag="wo")
            bo_sb = pp.tile([128, NKD], F32, tag="bo")

            # ====== Phases A (projections), B (attention), C (out proj) ======
            # emitted interleaved (A0 A1 B0 C0 A2 B1 C1 A3 B2 C2 B3 C3) so the
            # exp-heavy (ACT-bound) B sections overlap the PE-bound A sections.
            # One unified PSUM pool: A's q/k accumulators share the "st" tag,
            # A's v accumulators share the "fo" tag with C's output tiles.


            def phase_a(c):
                xs = []
                for k in range(NKD):
                    t = xp.tile([128, CH], F32R, tag=f"xs{k}", name=f"xs{k}_{c}")
                    eng = nc.sync if k % 2 == 0 else nc.scalar
                    eng.dma_start(
                        out=t[:], in_=xT.ap()[k * 128 : (k + 1) * 128, c * CH : (c + 1) * CH]
                    )
                    xs.append(t)
                    if c == 0:
                        nc.sync.dma_start(out=wq_sb[:, k, :], in_=wq_ap[:, k, :])
                        nc.gpsimd.dma_start(out=wk_sb[:, k, :], in_=wk_ap[:, k, :])
                if c == 0:
                    for k in range(NKD):
                        nc.gpsimd.dma_start(out=wv_sb[:, k, :], in_=wv_ap[:, k, :])
                for name, w_sb, dst, scale in (
                    ("q", wq_sb, qt_sb, None),
                    ("k", wk_sb, kt_sb, 0.125),
                ):
                    for m in range(2):
                        ps_t = ps.tile([128, CH], F32, tag="st", bufs=4, name=f"{name}ps{m}_{c}")
                        for k in range(NKD):
                            nc.tensor.matmul(
                                ps_t[:],
                                w_sb[:, k, m * 128 : (m + 1) * 128],
                                xs[k][:],
                                start=(k == 0),
                                stop=(not has_bias and k == NKD - 1),
                            )
                        if has_bias:
                            b_sb = bq_sb if name == "q" else bk_sb
                            nc.tensor.matmul(
                                ps_t[:],
                                b_sb[:, m * 128 : (m + 1) * 128],
                                ones_sb[:],
                                start=False,
                                stop=True,
                            )
                        dslice = dst[m][:, c * CH : (c + 1) * CH]
                        if scale is None:
                            nc.vector.tensor_copy(dslice, ps_t[:])
                        else:
                            nc.vector.tensor_scalar_mul(dslice, ps_t[:], scale)
                for i in range(4):
                    si = 4 * c + i
                    ps_t = ps.tile([128, DH], F32, tag="fo", bufs=2, name=f"vps_{si}")
                    for k in range(NKD):
                        nc.tensor.matmul(
                            ps_t[:],
                            xs[k][:, i * 128 : (i + 1) * 128],
                            wv_sb[:, k, :],
                            start=(k == 0),
                            stop=(not has_bias and k == NKD - 1),
                        )
                    if has_bias:
                        nc.tensor.matmul(
                            ps_t[:], ones_sb[:, 0:128], bv_sb[:], start=False, stop=True
                        )
                    nc.vector.tensor_copy(
                        vext[si][:, :, 0:DK],
                        ps_t[:].rearrange("p (h d) -> p h d", h=HPC),
                    )
                if c == 0:
                    emit_late_dmas()

            def phase_b(c):
                nk = 4 * c + 4  # causal: sk-tiles 0 .. 4c+3
                for hp in (0, 2):  # interleave two heads per stream
                    pair = (hp, hp + 1)
                    avs = {}
                    for h in pair:
                        avs[h] = ps.tile(
                            [DK + 1, CH], F32, tag=f"av{h % 2}", bufs=1, name=f"av{h}_{c}"
                        )
                    for t in range(nk):
                        o = t - 4 * c
                        pts = {}
                        for h in pair:
                            m, r0 = h // 2, (h % 2) * DK
                            st = ps.tile(
                                [128, CH], F32, tag="st", bufs=4, name=f"st{h}_{c}_{t}"
                            )
                            nc.tensor.matmul(
                                st[:],
                                kt_sb[m][r0 : r0 + DK, t * 128 : (t + 1) * 128],
                                qt_sb[m][r0 : r0 + DK, c * CH : (c + 1) * CH],
                                start=True,
                                stop=True,
                            )
                            pt = ptp.tile([128, CH], F32R, tag="pt", name=f"pt{h}_{c}_{t}")
                            if o >= 0:
                                nc.vector.tensor_add(
                                    st[:, 128 * o : 128 * (o + 1)],
                                    st[:, 128 * o : 128 * (o + 1)],
                                    mask_sb[:],
                                )
                            z = 128 * max(o, 0)
                            nc.scalar.activation(
                                out=pt[:, z:CH], in_=st[:, z:CH], func=Exp
                            )
                            pts[h] = (pt, z)
                        for h in pair:
                            pt, z = pts[h]
                            nc.tensor.matmul(
                                avs[h][:, z:CH],
                                vext[t][:, h, :],
                                pt[:, z:CH],
                                start=(t == 0),
                                stop=(t == nk - 1),
                            )
                    for h in pair:
                        m, r0 = h // 2, (h % 2) * DK
                        av = avs[h]
                        recip = sp.tile([1, CH], F32, tag="recip", name=f"recip{h}_{c}")
                        nc.vector.reciprocal(recip[:], av[DK : DK + 1, :])
                        bc_sb = sp.tile([DK, CH], F32, tag="bcsb", name=f"bcsb{h}_{c}")
                        nc.gpsimd.partition_broadcast(bc_sb[:], recip[:])
                        nc.vector.tensor_mul(
                            ctx_sb[m][r0 : r0 + DK, c * CH : (c + 1) * CH],
                            av[0:DK, :],
                            bc_sb[:],
                        )

            def phase_c(c):
                for dt in range(NKD):
                    fo = ps.tile([128, CH], F32, tag="fo", bufs=2, name=f"fo{dt}_{c}")
                    for kk in range(2):
                        nc.tensor.matmul(
                            fo[:],
                            wo_sb[:, kk, dt * 128 : (dt + 1) * 128],
                            ctx_sb[kk][:, c * CH : (c + 1) * CH],
                            start=(kk == 0),
                            stop=(kk == 1),
                        )
                    ob = wp.tile([128, CH], F32, tag="ob", name=f"ob{dt}_{c}")
                    nc.vector.tensor_scalar_add(ob[:], fo[:], bo_sb[:, dt : dt + 1])
                    nc.scalar.dma_start(
                        out=outT.ap()[dt * 128 : (dt + 1) * 128, c * CH : (c + 1) * CH],
                        in_=ob[:],
                    )

            phase_a(0)
            phase_a(1)
            for c in range(NCH):
                if c + 2 < NCH:
                    phase_a(c + 2)
                phase_b(c)
                phase_c(c)


    nc.finalize()
    return nc


_NC_CACHE = {}


def get_nc(has_bias: bool = True):
    key = ("nc", has_bias)
    if key not in _NC_CACHE:
        _NC_CACHE[key] = build_program(has_bias)
    return _NC_CACHE[key]


def make_in_maps(x, mask, Wq, bq, Wk, bk, Wv, bv, Wo, bo):
    x = np.asarray(x, dtype=np.float32)
    mask = np.asarray(mask)
    Wq, Wk, Wv, Wo = (np.asarray(w, dtype=np.float32) for w in (Wq, Wk, Wv, Wo))
    bq, bk, bv, bo = (np.asarray(v, dtype=np.float32) for v in (bq, bk, bv, bo))

    # lower-triangle-complement boundary strip: masked iff key j > query i
    # within a diagonal 128x128 block -> additive MASK_NEG where mask==1
    trimask = np.where(mask[0:128, 0:128].T == 1, MASK_NEG, 0.0).astype(np.float32)
    ones_row = np.ones((1, CH), dtype=np.float32)
    ones4 = np.ones((128, HPC, 1), dtype=np.float32)
    zeros_bo = np.zeros_like(bo)

    in_maps = []
    for core in range(NCORES):
        b, g = core // 4, core % 4
        sl = slice(g * DH, (g + 1) * DH)
        in_maps.append(
            {
                "xT": np.ascontiguousarray(x[b].T),
                "wq": np.ascontiguousarray(Wq[:, sl]),
                "wk": np.ascontiguousarray(Wk[:, sl]),
                "wv": np.ascontiguousarray(Wv[:, sl]),
                "wo": np.ascontiguousarray(Wo[sl, :]),
                "bq": np.ascontiguousarray(bq[sl]),
                "bk": np.ascontiguousarray(bk[sl]),
                "bv": np.ascontiguousarray(bv[sl]),
                "bo": bo if g == 0 else zeros_bo,
                "trimask": trimask,
                "ones_row": ones_row,
                "ones4": ones4,
            }
        )
    return in_maps


def gather_output(results):
    out = np.empty((B, S, D), dtype=np.float32)
    for b in range(B):
        acc = results[b * 4]["outT"].copy()
        for g in range(1, 4):
            acc += results[b * 4 + g]["outT"]
        out[b] = acc.T
    return out


def _get_runner(has_bias):
    """Compile the SPMD program once per process; reuse across kernel() calls."""
    key = ("runner", has_bias)
    if key in _NC_CACHE:
        return _NC_CACHE[key]
    import jax
    from concourse import bass2jax
    from concourse.bass2jax import _bass_exec_p, install_neuronx_cc_hook, partition_id_tensor

    nc = get_nc(has_bias)
    install_neuronx_cc_hook()
    partition_name = nc.partition_id_tensor.name if nc.partition_id_tensor else None
    in_names, out_names, out_avals, zero_outs = [], [], [], []
    for alloc in nc.m.functions[0].allocations:
        if not isinstance(alloc, mybir.MemoryLocationSet):
            continue
        name = alloc.memorylocations[0].name
        if alloc.kind == "ExternalInput":
            if name != partition_name:
                in_names.append(name)
        elif alloc.kind == "ExternalOutput":
            shape = tuple(alloc.tensor_shape)
            dtype = mybir.dt.np(alloc.dtype)
            out_names.append(name)
            out_avals.append(jax.core.ShapedArray(shape, dtype))
            zero_outs.append(np.zeros(shape, dtype))
    n_params = len(in_names)
    all_in_names = in_names + out_names
    if partition_name is not None:
        all_in_names = all_in_names + [partition_name]

    def _body(*args):
        operands = list(args)
        if partition_name is not None:
            operands.append(partition_id_tensor())
        outs = _bass_exec_p.bind(
            *operands,
            out_avals=tuple(out_avals),
            in_names=tuple(all_in_names),
            out_names=tuple(out_names),
            lowering_input_output_aliases=(),
            sim_require_finite=True,
            sim_require_nnan=True,
            nc=nc,
        )
        return tuple(outs)

    devices = jax.devices()[:NCORES]
    mesh = bass2jax.Mesh(np.asarray(devices), ("core",))
    P = bass2jax.PartitionSpec
    sharded = jax.jit(
        bass2jax.shard_map(
            _body,
            mesh=mesh,
            in_specs=(P("core"),) * (n_params + len(out_names)),
            out_specs=(P("core"),) * len(out_names),
            check_rep=False,
        ),
        keep_unused=True,
    )
    concat_zeros = [
        np.zeros((NCORES * z.shape[0], *z.shape[1:]), z.dtype) for z in zero_outs
    ]

    def run(in_maps):
        concat_in = [
            np.concatenate([np.asarray(in_maps[c][nm]) for c in range(NCORES)], axis=0)
            for nm in in_names
        ]
        outs = sharded(*concat_in, *concat_zeros)
        return [
            {
                nm: np.asarray(outs[i]).reshape(NCORES, *out_avals[i].shape)[c]
                for i, nm in enumerate(out_names)
            }
            for c in range(NCORES)
        ]

    _NC_CACHE[key] = run
    return run


def kernel(x, mask, Wq, bq, Wk, bk, Wv, bv, Wo, bo):
    has_bias = any(
        np.asarray(v, dtype=np.float32).any() for v in (bq, bk, bv)
    )
    in_maps = make_in_maps(x, mask, Wq, bq, Wk, bk, Wv, bv, Wo, bo)
    results = _get_runner(has_bias)(in_maps)
    return gather_output(results)


# revision 34
# speedup vs baseline: 1.6472x; 1.5844x over previous
"""Multi-head attention (B=2, S=2048, D=1024, H=16) on 8 NeuronCores.

Sharding: core = b*4 + g  (b = batch 0..1, g = head-group 0..3, 4 heads each).
Per core, everything is computed in transposed layouts so no on-device
transposes are needed:
  qT,kT = W.T @ x.T   (via matmul lhsT=W, rhs=xT)            [256, 2048]
  v     = x @ Wv      (normal layout, + ones column)          [2048, 4, 65]
  ST    = kT.T @ qT   (scores, transposed: [sk, sq] tiles)
  P^T   = exp(ST + tri-mask)      (elementwise, no transpose needed)
  outT  = [v|1].T @ P^T           (ones column -> softmax sums for free)
  ctxT  = outT[0:64] * (1/outT[64])   (broadcast via PE ones-matmul)
  yT    = Wo.T-slice @ ctxT + bo  (partial over head group)   [1024, 2048]
Host sums the 4 partial yT per batch and transposes.

Matmuls run as float32r (fp32 bits, TF32-like PE fast path, ~1.5e-4 rel err).
Causal masking: only sk-tiles t <= 4c+3 are computed; the diagonal tile's
boundary strip gets an additive lower-triangle mask on DVE, fully-masked
column prefixes are memset to 0 on GpSimd and skipped by the exp.
"""

import numpy as np

import concourse.mybir as mybir
import concourse.tile as tile
from concourse import bacc

B, S, D, H = 2, 2048, 1024, 16
DK = 64
NCORES = 8
HPC = 4                      # heads per core
DH = HPC * DK                # 256 per-core head dims
CH = 512                     # sq chunk size
NCH = S // CH                # 4 chunks
NST = S // 128               # 16 s-tiles
NKD = D // 128               # 8 contraction tiles over D
F32 = mybir.dt.float32
F32R = mybir.dt.float32r
MASK_NEG = -100.0


def build_program(has_bias: bool):
    nc = bacc.Bacc("TRN2", target_bir_lowering=False, debug=False)

    xT = nc.dram_tensor("xT", [D, S], F32R, kind="ExternalInput")
    wq = nc.dram_tensor("wq", [D, DH], F32R, kind="ExternalInput")
    wk = nc.dram_tensor("wk", [D, DH], F32R, kind="ExternalInput")
    wv = nc.dram_tensor("wv", [D, DH], F32R, kind="ExternalInput")
    wo = nc.dram_tensor("wo", [DH, D], F32R, kind="ExternalInput")
    bq = nc.dram_tensor("bq", [DH], F32R, kind="ExternalInput")
    bk = nc.dram_tensor("bk", [DH], F32R, kind="ExternalInput")
    bv = nc.dram_tensor("bv", [DH], F32R, kind="ExternalInput")
    bo = nc.dram_tensor("bo", [D], F32, kind="ExternalInput")
    trimask = nc.dram_tensor("trimask", [128, 128], F32, kind="ExternalInput")
    ones_row = nc.dram_tensor("ones_row", [1, CH], F32R, kind="ExternalInput")
    ones4 = nc.dram_tensor("ones4", [128, HPC, 1], F32R, kind="ExternalInput")
    outT = nc.dram_tensor("outT", [D, S], F32, kind="ExternalOutput")

    Exp = mybir.ActivationFunctionType.Exp
    Copy = mybir.ActivationFunctionType.Copy

    with tile.TileContext(nc) as tc:
        with (
            tc.tile_pool(name="persist", bufs=1) as pp,
            tc.tile_pool(name="work", bufs=6) as wp,
            tc.tile_pool(name="pt_pool", bufs=6) as ptp,
            tc.tile_pool(name="small", bufs=6) as sp,
            tc.tile_pool(name="xsl", bufs=2) as xp,
            tc.tile_pool(name="psum", bufs=2, space="PSUM") as ps,
        ):
            # ---- SBUF tensors needed by phase A chunk 0, DMA'd first ----
            wq_sb = pp.tile([128, NKD, DH], F32R, tag="wq")
            wk_sb = pp.tile([128, NKD, DH], F32R, tag="wk")
            wv_sb = pp.tile([128, NKD, DH], F32R, tag="wv")
            wq_ap = wq.ap().rearrange("(t p) n -> p t n", p=128)
            wk_ap = wk.ap().rearrange("(t p) n -> p t n", p=128)
            wv_ap = wv.ap().rearrange("(t p) n -> p t n", p=128)

            qt_sb = [pp.tile([128, S], F32R, tag=f"qt{m}", name=f"qt{m}") for m in range(2)]
            kt_sb = [pp.tile([128, S], F32R, tag=f"kt{m}", name=f"kt{m}") for m in range(2)]
            ctx_sb = [pp.tile([128, S], F32R, tag=f"ctx{m}", name=f"ctx{m}") for m in range(2)]
            vext = [pp.tile([128, HPC, DK + 1], F32R, tag=f"vext{i}", name=f"vext{i}") for i in range(NST)]

            ones_sb = pp.tile([1, CH], F32R, tag="ones")
            if has_bias:
                bq_sb = pp.tile([1, DH], F32R, tag="bq")
                bk_sb = pp.tile([1, DH], F32R, tag="bk")
                bv_sb = pp.tile([1, DH], F32R, tag="bv")

            def emit_late_dmas():
                """Inputs not needed until later; emitted after chunk 0."""
                nc.gpsimd.dma_start(out=mask_sb[:], in_=trimask.ap())
                nc.gpsimd.dma_start(out=wo_sb[:], in_=wo.ap().rearrange("(t p) n -> p t n", p=128))
                nc.gpsimd.dma_start(out=bo_sb[:], in_=bo.ap().rearrange("(t p) -> p t", p=128))
                nc.gpsimd.dma_start(out=ones_sb[:], in_=ones_row.ap())
                if has_bias:
                    nc.gpsimd.dma_start(out=bq_sb[:], in_=bq.ap()[None, :])
                    nc.gpsimd.dma_start(out=bk_sb[:], in_=bk.ap()[None, :])
                    nc.gpsimd.dma_start(out=bv_sb[:], in_=bv.ap()[None, :])
                for i in range(NST):
                    nc.gpsimd.dma_start(out=vext[i][:, :, DK : DK + 1], in_=ones4.ap())

            mask_sb = pp.tile([128, 128], F32, tag="mask")
            wo_sb = pp.tile([128, 2, D], F32R, tag="wo")
            bo_sb = pp.tile([128, NKD], F32, tag="bo")

            # ====== Phases A (projections), B (attention), C (out proj) ======
            # emitted interleaved (A0 A1 B0 C0 A2 B1 C1 A3 B2 C2 B3 C3) so the
            # exp-heavy (ACT-bound) B sections overlap the PE-bound A sections.
            # One unified PSUM pool: A's q/k accumulators share the "st" tag,
            # A's v accumulators share the "fo" tag with C's output tiles.


            def phase_a(c):
                xs = []
                for k in range(NKD):
                    t = xp.tile([128, CH], F32R, tag=f"xs{k}", name=f"xs{k}_{c}")
                    eng = nc.sync if k % 2 == 0 else nc.scalar
                    eng.dma_start(
                        out=t[:], in_=xT.ap()[k * 128 : (k + 1) * 128, c * CH : (c + 1) * CH]
                    )
                    xs.append(t)
                    if c == 0:
                        nc.sync.dma_start(out=wq_sb[:, k, :], in_=wq_ap[:, k, :])
                        nc.gpsimd.dma_start(out=wk_sb[:, k, :], in_=wk_ap[:, k, :])
                if c == 0:
                    for k in range(NKD):
                        nc.gpsimd.dma_start(out=wv_sb[:, k, :], in_=wv_ap[:, k, :])
                for name, w_sb, dst, scale in (
                    ("q", wq_sb, qt_sb, None),
                    ("k", wk_sb, kt_sb, 0.125),
                ):
                    for m in range(2):
                        ps_t = ps.tile([128, CH], F32, tag="st", bufs=4, name=f"{name}ps{m}_{c}")
                        for k in range(NKD):
                            nc.tensor.matmul(
                                ps_t[:],
                                w_sb[:, k, m * 128 : (m + 1) * 128],
                                xs[k][:],
                                start=(k == 0),
                                stop=(not has_bias and k == NKD - 1),
                            )
                        if has_bias:
                            b_sb = bq_sb if name == "q" else bk_sb
                            nc.tensor.matmul(
                                ps_t[:],
                                b_sb[:, m * 128 : (m + 1) * 128],
                                ones_sb[:],
                                start=False,
                                stop=True,
                            )
                        dslice = dst[m][:, c * CH : (c + 1) * CH]
                        if scale is None:
                            nc.vector.tensor_copy(dslice, ps_t[:])
                        else:
                            nc.vector.tensor_scalar_mul(dslice, ps_t[:], scale)
                for i in range(4):
                    si = 4 * c + i
                    ps_t = ps.tile([128, DH], F32, tag="fo", bufs=2, name=f"vps_{si}")
                    for k in range(NKD):
                        nc.tensor.matmul(
                            ps_t[:],
                            xs[k][:, i * 128 : (i + 1) * 128],
                            wv_sb[:, k, :],
                            start=(k == 0),
                            stop=(not has_bias and k == NKD - 1),
                        )
                    if has_bias:
                        nc.tensor.matmul(
                            ps_t[:], ones_sb[:, 0:128], bv_sb[:], start=False, stop=True
                        )
                    nc.vector.tensor_copy(
                        vext[si][:, :, 0:DK],
                        ps_t[:].rearrange("p (h d) -> p h d", h=HPC),
                    )
                if c == 0:
                    emit_late_dmas()

            def phase_b(c):
                nk = 4 * c + 4  # causal: sk-tiles 0 .. 4c+3
                for hp in (0, 2):  # interleave two heads per stream
                    pair = (hp, hp + 1)
                    avs = {}
                    for h in pair:
                        avs[h] = ps.tile(
                            [DK + 1, CH], F32, tag=f"av{h % 2}", bufs=1, name=f"av{h}_{c}"
                        )
                    for t in range(nk):
                        o = t - 4 * c
                        pts = {}
                        for h in pair:
                            m, r0 = h // 2, (h % 2) * DK
                            st = ps.tile(
                                [128, CH], F32, tag="st", bufs=4, name=f"st{h}_{c}_{t}"
                            )
                            nc.tensor.matmul(
                                st[:],
                                kt_sb[m][r0 : r0 + DK, t * 128 : (t + 1) * 128],
                                qt_sb[m][r0 : r0 + DK, c * CH : (c + 1) * CH],
                                start=True,
                                stop=True,
                            )
                            pt = ptp.tile([128, CH], F32R, tag="pt", name=f"pt{h}_{c}_{t}")
                            if o >= 0:
                                nc.vector.tensor_add(
                                    st[:, 128 * o : 128 * (o + 1)],
                                    st[:, 128 * o : 128 * (o + 1)],
                                    mask_sb[:],
                                )
                            z = 128 * max(o, 0)
                            nc.scalar.activation(
                                out=pt[:, z:CH], in_=st[:, z:CH], func=Exp
                            )
                            pts[h] = (pt, z)
                        for h in pair:
                            pt, z = pts[h]
                            nc.tensor.matmul(
                                avs[h][:, z:CH],
                                vext[t][:, h, :],
                                pt[:, z:CH],
                                start=(t == 0),
                                stop=(t == nk - 1),
                            )
                    for h in pair:
                        m, r0 = h // 2, (h % 2) * DK
                        av = avs[h]
                        recip = sp.tile([1, CH], F32, tag="recip", name=f"recip{h}_{c}")
                        nc.vector.reciprocal(recip[:], av[DK : DK + 1, :])
                        bc_sb = sp.tile([DK, CH], F32, tag="bcsb", name=f"bcsb{h}_{c}")
                        nc.gpsimd.partition_broadcast(bc_sb[:], recip[:])
                        nc.vector.tensor_mul(
                            ctx_sb[m][r0 : r0 + DK, c * CH : (c + 1) * CH],
                            av[0:DK, :],
                            bc_sb[:],
                        )

            def phase_c(c):
                for dt in range(NKD):
                    fo = ps.tile([128, CH], F32, tag="fo", bufs=2, name=f"fo{dt}_{c}")
                    for kk in range(2):
                        nc.tensor.matmul(
                            fo[:],
                            wo_sb[:, kk, dt * 128 : (dt + 1) * 128],
                            ctx_sb[kk][:, c * CH : (c + 1) * CH],
                            start=(kk == 0),
                            stop=(kk == 1),
                        )
                    ob = wp.tile([128, CH], F32, tag="ob", name=f"ob{dt}_{c}")
                    nc.vector.tensor_scalar_add(ob[:], fo[:], bo_sb[:, dt : dt + 1])
                    nc.scalar.dma_start(
                        out=outT.ap()[dt * 128 : (dt + 1) * 128, c * CH : (c + 1) * CH],
                        in_=ob[:],
                    )

            phase_a(0)
            phase_a(1)
            for c in range(NCH):
                if c + 2 < NCH:
                    phase_a(c + 2)
                phase_b(c)
                phase_c(c)


    nc.finalize()
    return nc


_NC_CACHE = {}


def get_nc(has_bias: bool = True):
    key = ("nc", has_bias)
    if key not in _NC_CACHE:
        _NC_CACHE[key] = build_program(has_bias)
    return _NC_CACHE[key]


def make_in_maps(x, mask, Wq, bq, Wk, bk, Wv, bv, Wo, bo):
    x = np.asarray(x, dtype=np.float32)
    mask = np.asarray(mask)
    Wq, Wk, Wv, Wo = (np.asarray(w, dtype=np.float32) for w in (Wq, Wk, Wv, Wo))
    bq, bk, bv, bo = (np.asarray(v, dtype=np.float32) for v in (bq, bk, bv, bo))

    # lower-triangle-complement boundary strip: masked iff key j > query i
    # within a diagonal 128x128 block -> additive MASK_NEG where mask==1
    trimask = np.where(mask[0:128, 0:128].T == 1, MASK_NEG, 0.0).astype(np.float32)
    ones_row = np.ones((1, CH), dtype=np.float32)
    ones4 = np.ones((128, HPC, 1), dtype=np.float32)
    zeros_bo = np.zeros_like(bo)

    in_maps = []
    for core in range(NCORES):
        b, g = core // 4, core % 4
        sl = slice(g * DH, (g + 1) * DH)
        in_maps.append(
            {
                "xT": np.ascontiguousarray(x[b].T),
                "wq": np.ascontiguousarray(Wq[:, sl]),
                "wk": np.ascontiguousarray(Wk[:, sl]),
                "wv": np.ascontiguousarray(Wv[:, sl]),
                "wo": np.ascontiguousarray(Wo[sl, :]),
                "bq": np.ascontiguousarray(bq[sl]),
                "bk": np.ascontiguousarray(bk[sl]),
                "bv": np.ascontiguousarray(bv[sl]),
                "bo": bo if g == 0 else zeros_bo,
                "trimask": trimask,
                "ones_row": ones_row,
                "ones4": ones4,
            }
        )
    return in_maps


def gather_output(results):
    out = np.empty((B, S, D), dtype=np.float32)
    for b in range(B):
        acc = results[b * 4]["outT"].copy()
        for g in range(1, 4):
            acc += results[b * 4 + g]["outT"]
        out[b] = acc.T
    return out


def _get_runner(has_bias):
    """Compile the SPMD program once per process; reuse across kernel() calls."""
    key = ("runner", has_bias)
    if key in _NC_CACHE:
        return _NC_CACHE[key]
    import jax
    from concourse import bass2jax
    from concourse.bass2jax import _bass_exec_p, install_neuronx_cc_hook, partition_id_tensor

    nc = get_nc(has_bias)
    install_neuronx_cc_hook()
    partition_name = nc.partition_id_tensor.name if nc.partition_id_tensor else None
    in_names, out_names, out_avals, zero_outs = [], [], [], []
    for alloc in nc.m.functions[0].allocations:
        if not isinstance(alloc, mybir.MemoryLocationSet):
            continue
        name = alloc.memorylocations[0].name
        if alloc.kind == "ExternalInput":
            if name != partition_name:
                in_names.append(name)
        elif alloc.kind == "ExternalOutput":
            shape = tuple(alloc.tensor_shape)
            dtype = mybir.dt.np(alloc.dtype)
            out_names.append(name)
            out_avals.append(jax.core.ShapedArray(shape, dtype))
            zero_outs.append(np.zeros(shape, dtype))
    n_params = len(in_names)
    all_in_names = in_names + out_names
    if partition_name is not None:
        all_in_names = all_in_names + [partition_name]

    def _body(*args):
        operands = list(args)
        if partition_name is not None:
            operands.append(partition_id_tensor())
        outs = _bass_exec_p.bind(
            *operands,
            out_avals=tuple(out_avals),
            in_names=tuple(all_in_names),
            out_names=tuple(out_names),
            lowering_input_output_aliases=(),
            sim_require_finite=True,
            sim_require_nnan=True,
            nc=nc,
        )
        return tuple(outs)

    devices = jax.devices()[:NCORES]
    mesh = bass2jax.Mesh(np.asarray(devices), ("core",))
    P = bass2jax.PartitionSpec
    sharded = jax.jit(
        bass2jax.shard_map(
            _body,
            mesh=mesh,
            in_specs=(P("core"),) * (n_params + len(out_names)),
            out_specs=(P("core"),) * len(out_names),
            check_rep=False,
        ),
        keep_unused=True,
    )
    concat_zeros = [
        np.zeros((NCORES * z.shape[0], *z.shape[1:]), z.dtype) for z in zero_outs
    ]

    def run(in_maps):
        concat_in = [
            np.concatenate([np.asarray(in_maps[c][nm]) for c in range(NCORES)], axis=0)
            for nm in in_names
        ]
        outs = sharded(*concat_in, *concat_zeros)
        return [
            {
                nm: np.asarray(outs[i]).reshape(NCORES, *out_avals[i].shape)[c]
                for i, nm in enumerate(out_names)
            }
            for c in range(NCORES)
        ]

    _NC_CACHE[key] = run
    return run


def kernel(x, mask, Wq, bq, Wk, bk, Wv, bv, Wo, bo):
    has_bias = any(
        np.asarray(v, dtype=np.float32).any() for v in (bq, bk, bv)
    )
    in_maps = make_in_maps(x, mask, Wq, bq, Wk, bk, Wv, bv, Wo, bo)
    results = _get_runner(has_bias)(in_maps)
    return gather_output(results)
